# revision 18
# baseline (speedup 1.0000x reference)
"""Averaged Hausdorff loss on 8 Trainium2 NeuronCores.

Problem: set1, set2 [B=4, N=4096, D=3] fp32.
  dist[b, n, m] = ||set1[b,n] - set2[b,m]||
  out = mean_b( mean_n min_m dist + mean_m min_n dist )

Sharding: one core per (batch, orientation) pair -> exactly 8 cores.
  core 2b+0: row mins  (queries = set1[b], database = set2[b])
  core 2b+1: col mins  (queries = set2[b], database = set1[b])

v3 kernel (per core): with q = query point, s = database point,
  d2(q, s) = ||q||^2 + (||s||^2 - 2 q.s)
The parenthesized part is ONE K=11 fp16 matmul: matmul cost on the PE is
(moving columns) x (cycles/column) regardless of K, so the three hi/lo
precision passes of the old kernel (xh.uh + xh.ul + xl.uh, each K=4) stack
vertically into a single K=11 instruction at one third the PE time:
  lhsT rows = [qh, qh, ql, 1, 1]          (fp16 hi/lo split of q, 3+3+3+1+1)
  rhs  rows = [uh, ul, uh, s2h, s2l]      (u = -2 s, s2 = ||s||^2)
fp16 x fp16 products are exact in fp32 and the dropped xl.ul term is
~2^-22 relative, so the distance matrix is fp32-grade (measured final
rel err ~5e-5).

Reduction: per (query-tile, half-db) unit the four PSUM banks are drained at
the joint PSUM-read roofline of ScalarE + VectorE: ScalarE copies two banks
to SBUF scratch (its 1.2 elem/ns is the fastest PSUM drain), VectorE then
issues ONE fused tensor_tensor_reduce(min, min) whose in0 is the other two
PSUM banks and in1 the staged pair -- consuming two fresh elements per DVE
cycle and emitting the unit's running row-min straight into a [128,1] accum
column (no scan-tail extraction).
"""

import os
import sys

import numpy as np

for _p in ("/opt/trn_rl_repo",):
    if _p not in sys.path and os.path.isdir(_p):
        sys.path.insert(0, _p)

B, N, D = 4, 4096, 3
NCORES = 8
NTILES = N // 128          # 32 query tiles of 128
NCHUNKS = N // 512         # 8 database chunks of 512
KROWS = 16                 # 11 used contraction rows, padded to 16
VERSION = 4
NUNITS3 = NTILES * 2       # 64 (tile, half-db) units
NUNITS5 = NTILES * 4       # 128 (tile, quarter-db) units
FMAX = 3.0e38

_nc_cache = None


def _build_nc_v5(loop_iters: int | None = None):
    """v5: 2-bank units with 4-deep PSUM rotation.

    Each unit is one (query-tile, db-quarter): two strip-rotated matmuls
    (one bank ScalarE-staged, one scanned directly from PSUM), one 512-free
    VectorE scan. With 4 units in flight (8 banks), every cross-engine
    semaphore wait is satisfied several periods in advance, so the measured
    ~0.3-0.7us semaphore propagation latencies hide completely and the DVE
    runs back-to-back at its measured 893.5 ns/scan floor. The scan tails
    land in an 8-slot junk ring; ScalarE extracts pairs of tails with one
    strided copy each, ordered so extracts always trail the scans they read
    by several units.
    """
    import concourse.bass as bass
    from concourse import mybir
    from contextlib import ExitStack

    nc = bass.Bass("TRN2", target_bir_lowering=False, debug=False,
                   num_devices=NCORES)
    WR = nc.dram_tensor("WR", [KROWS, 2 * N], mybir.dt.float16,
                        kind="ExternalInput").ap()
    OUT = nc.dram_tensor("OUT", [128, NUNITS5], mybir.dt.float32,
                         kind="ExternalOutput").ap()

    ctx = ExitStack()
    with ctx:
        wr = ctx.enter_context(
            nc.sbuf_tensor("wr_sb", [128, 2 * N], mybir.dt.float16)).ap()
        mins = ctx.enter_context(
            nc.sbuf_tensor("mins_sb", [128, NUNITS5], mybir.dt.float32)).ap()
        scratch = ctx.enter_context(
            nc.sbuf_tensor("scr_sb", [128, 4, 512], mybir.dt.float32)).ap()
        junk = ctx.enter_context(
            nc.sbuf_tensor("junk_sb", [128, 8, 512], mybir.dt.float32)).ap()
        psum = ctx.enter_context(
            nc.psum_tensor("psum", [128, N], mybir.dt.float32)).ap()
        dma_sem = ctx.enter_context(nc.semaphore("dma_sem"))
        pe_sem = ctx.enter_context(nc.semaphore("pe_sem"))
        act_sem = ctx.enter_context(nc.semaphore("act_sem"))
        vec_sem = ctx.enter_context(nc.semaphore("vec_sem"))

        def w_ap(s, t):
            return wr[32 * s:32 * s + 11, t * 128:(t + 1) * 128]

        def r_ap(s, c):
            return wr[32 * s:32 * s + 11, N + c * 512:N + (c + 1) * 512]

        for s in range(4):
            nc.sync.dma_start(out=wr[32 * s:32 * s + KROWS, :],
                              in_=WR[:]).then_inc(dma_sem, 16)

        def emit_body():
            nc.tensor.wait_ge(dma_sem, 64)
            stage_idx = {}
            n_act = 0
            n_extract = 0

            def emit_extract(k):
                # tails of units 2k, 2k+1 (junk slots (2k)%8, (2k)%8+1)
                j0 = (2 * k) % 8
                nc.scalar.copy(mins[:, 2 * k:2 * k + 2],
                               junk[:, j0:j0 + 2, 511]
                               ).then_inc(act_sem, 1)

            for u in range(NUNITS5):
                t, q = u // 4, u % 4
                slot = u % 4
                bank_s = 2 * slot + 1
                bank_p = 2 * slot
                # staged-bank matmul (even strip): its bank was drained by
                # stage(u-4).
                if u >= 4:
                    nc.tensor.wait_ge(act_sem, stage_idx[u - 4])
                mm = nc.tensor.matmul(
                    psum[:, 512 * bank_s:512 * (bank_s + 1)],
                    w_ap((2 * u) % 4, t), r_ap((2 * u) % 4, 2 * q + 1),
                    start=True, stop=True,
                    tile_position=(32 * ((2 * u) % 4), 0))
                mm.then_inc(pe_sem, 1)
                # scan-bank matmul (odd strip): its bank was read by scan(u-4)
                if u >= 4:
                    nc.tensor.wait_ge(vec_sem, u - 3)
                mm = nc.tensor.matmul(
                    psum[:, 512 * bank_p:512 * (bank_p + 1)],
                    w_ap((2 * u + 1) % 4, t), r_ap((2 * u + 1) % 4, 2 * q),
                    start=True, stop=True,
                    tile_position=(32 * ((2 * u + 1) % 4), 0))
                mm.then_inc(pe_sem, 1)

                # ScalarE: staged bank -> scratch slot
                nc.scalar.wait_ge(pe_sem, 2 * u + 1)
                if u >= 4:
                    nc.scalar.wait_ge(vec_sem, u - 3)  # scratch WAR
                nc.scalar.copy(scratch[:, slot, :],
                               psum[:, 512 * bank_s:512 * (bank_s + 1)]
                               ).then_inc(act_sem, 1)
                n_act += 1
                stage_idx[u] = n_act
                # Extract tails for pair k once scan(2k+1) is implied by this
                # stage's own vec wait (u - 3 >= 2k + 2  =>  k = (u-5)//2).
                if u >= 5 and u % 2 == 1:
                    emit_extract((u - 5) // 2)
                    n_act += 1
                    n_extract += 1

                # VectorE: one 512-free running-min scan; junk-ring WAR vs
                # the extract that read slot u%8 (emitted after stage(u-3)).
                nc.vector.wait_ge(pe_sem, 2 * u + 2)
                nc.vector.wait_ge(act_sem, stage_idx[u])
                nc.vector.tensor_tensor_scan(
                    out=junk[:, u % 8, :],
                    data0=psum[:, 512 * bank_p:512 * (bank_p + 1)],
                    data1=scratch[:, slot, :],
                    initial=FMAX,
                    op0=mybir.AluOpType.min, op1=mybir.AluOpType.min,
                ).then_inc(vec_sem, 1)

            while n_extract < NUNITS5 // 2:
                k = n_extract
                nc.scalar.wait_ge(vec_sem, 2 * k + 2)
                emit_extract(k)
                n_act += 1
                n_extract += 1
            return n_act

        if loop_iters is None:
            total_act = emit_body()
            nc.sync.wait_ge(act_sem, total_act)
        else:
            with nc.Fori(0, loop_iters):
                emit_body()
                nc.all_engine_barrier()
                nc.vector.sem_clear(pe_sem)
                nc.vector.sem_clear(act_sem)
                nc.vector.sem_clear(vec_sem)
                nc.all_engine_barrier()

        nc.sync.dma_start(out=OUT[:], in_=mins[:]).then_inc(dma_sem, 16)
        nc.sync.wait_ge(dma_sem, 80)

    return nc


def _build_nc_v4(loop_iters: int | None = None):
    """v4: like v3 but with the two hw-measured wins the cost model misses:

    1. Matmuls rotate across the four 32-row PE-array strips via
       tile_position=(32s, 0) (inputs replicated at partition groups
       0/32/64/96): strip-tiled matmuls overlap in the array, measured
       72 ns vs 441 ns per 512-col matmul.
    2. The DVE scan recurrence runs at ~2 cycles/element and its cost is
       superlinear in free size, so each unit issues two independent
       512-free scans (FMAX initial; chaining through `initial` costs
       +350 ns/op) and ScalarE extracts both tails with one strided copy.
    """
    import concourse.bass as bass
    from concourse import mybir
    from contextlib import ExitStack

    nc = bass.Bass("TRN2", target_bir_lowering=False, debug=False,
                   num_devices=NCORES)
    WR = nc.dram_tensor("WR", [KROWS, 2 * N], mybir.dt.float16,
                        kind="ExternalInput").ap()
    OUT = nc.dram_tensor("OUT", [128, 2 * NUNITS3], mybir.dt.float32,
                         kind="ExternalOutput").ap()

    ctx = ExitStack()
    with ctx:
        wr = ctx.enter_context(
            nc.sbuf_tensor("wr_sb", [128, 2 * N], mybir.dt.float16)).ap()
        mins = ctx.enter_context(
            nc.sbuf_tensor("mins_sb", [128, 2 * NUNITS3],
                           mybir.dt.float32)).ap()
        scratch = ctx.enter_context(
            nc.sbuf_tensor("scr_sb", [128, 4, 1024], mybir.dt.float32)).ap()
        junk = ctx.enter_context(
            nc.sbuf_tensor("junk_sb", [128, 4, 1024], mybir.dt.float32)).ap()
        psum = ctx.enter_context(
            nc.psum_tensor("psum", [128, N], mybir.dt.float32)).ap()
        dma_sem = ctx.enter_context(nc.semaphore("dma_sem"))
        pe_sem = ctx.enter_context(nc.semaphore("pe_sem"))
        act_sem = ctx.enter_context(nc.semaphore("act_sem"))
        vec_sem = ctx.enter_context(nc.semaphore("vec_sem"))

        def w_ap(s, t):
            return wr[32 * s:32 * s + 11, t * 128:(t + 1) * 128]

        def r_ap(s, c):
            return wr[32 * s:32 * s + 11, N + c * 512:N + (c + 1) * 512]

        for s in range(4):
            nc.sync.dma_start(out=wr[32 * s:32 * s + KROWS, :],
                              in_=WR[:]).then_inc(dma_sem, 16)

        def stage_idx(uu):
            # act-queue op index of stage(uu): units 0,1 emit only a stage;
            # units >= 2 emit [stage(u), extract(u-2)].
            return uu + 1 if uu < 2 else 2 * uu - 1

        def emit_extract(uu):
            # Both scan tails of unit uu (cols 511, 1023 of junk slot uu%4)
            # -> mins cols 2uu, 2uu+1, one strided ScalarE copy. Needs its
            # own vec wait (the surrounding stage's wait is 2 units weaker);
            # the act-queue stall this causes is absorbed by ScalarE's slack.
            nc.scalar.wait_ge(vec_sem, 2 * uu + 2)
            j = junk[:, uu % 4, :]
            nc.scalar.copy(mins[:, 2 * uu:2 * uu + 2],
                           j.rearrange("p (two f) -> p two f", two=2)[:, :, 511]
                           ).then_inc(act_sem, 1)

        def emit_body():
            nc.tensor.wait_ge(dma_sem, 64)
            n_act = 0
            for u in range(NUNITS3):
                t, h = u // 2, u % 2
                base = 2048 * h
                slot = u % 4
                # Staged pair (banks 2,3 / strips 2,3) first: they only
                # need stage(u-2) to have drained them.
                if u >= 2:
                    nc.tensor.wait_ge(act_sem, stage_idx(u - 2))
                for j in (2, 3):
                    mm = nc.tensor.matmul(
                        psum[:, base + 512 * j:base + 512 * (j + 1)],
                        w_ap(j, t), r_ap(j, 4 * h + j), start=True, stop=True,
                        tile_position=(32 * j, 0))
                mm.then_inc(pe_sem, 1)
                # Scan pair (banks 0,1 / strips 0,1): consumed by scans u-2.
                if u >= 2:
                    nc.tensor.wait_ge(vec_sem, 2 * u - 2)
                for j in (0, 1):
                    mm = nc.tensor.matmul(
                        psum[:, base + 512 * j:base + 512 * (j + 1)],
                        w_ap(j, t), r_ap(j, 4 * h + j), start=True, stop=True,
                        tile_position=(32 * j, 0))
                mm.then_inc(pe_sem, 1)

                # ScalarE: banks {2,3} of the quad -> scratch slot u%4. The
                # pe wait covers ALL four matmuls so the scan needs no pe
                # wait of its own; the 4-slot ring means the WAR is against
                # the scans of u-4 (two periods of slack).
                nc.scalar.wait_ge(pe_sem, 2 * u + 2)
                if u >= 4:
                    nc.scalar.wait_ge(vec_sem, 2 * u - 6)  # scratch WAR
                nc.scalar.copy(scratch[:, slot, :],
                               psum[:, base + 1024:base + 2048]
                               ).then_inc(act_sem, 1)
                n_act += 1
                if u >= 2:
                    emit_extract(u - 2)
                    n_act += 1

                # VectorE: two independent 512-free running-min scans. One
                # act wait: stage(u) implies all four matmuls; the junk-ring
                # WAR (extract u-4, emitted back at unit u-2) is also
                # covered by the same act count.
                nc.vector.wait_ge(act_sem, n_act)
                for k in range(2):
                    nc.vector.tensor_tensor_scan(
                        out=junk[:, slot, 512 * k:512 * (k + 1)],
                        data0=psum[:, base + 512 * k:base + 512 * (k + 1)],
                        data1=scratch[:, slot, 512 * k:512 * (k + 1)],
                        initial=FMAX,
                        op0=mybir.AluOpType.min, op1=mybir.AluOpType.min,
                    ).then_inc(vec_sem, 1)

            for uu in (NUNITS3 - 2, NUNITS3 - 1):
                emit_extract(uu)
                n_act += 1
            return n_act

        if loop_iters is None:
            total_act = emit_body()
            nc.sync.wait_ge(act_sem, total_act)
        else:
            with nc.Fori(0, loop_iters):
                emit_body()
                nc.all_engine_barrier()
                nc.vector.sem_clear(pe_sem)
                nc.vector.sem_clear(act_sem)
                nc.vector.sem_clear(vec_sem)
                nc.all_engine_barrier()

        nc.sync.dma_start(out=OUT[:], in_=mins[:]).then_inc(dma_sem, 16)
        nc.sync.wait_ge(dma_sem, 80)

    return nc


def _build_nc_v3(loop_iters: int | None = None):
    """Raw-Bass pipeline: PE streams one K=11 fp16 matmul per (tile, chunk)
    into a rotating half of PSUM; ScalarE stages the odd bank-pair of each
    4-bank unit to SBUF; VectorE fuses elementwise-min + min-reduce over
    (2 PSUM banks, 2 staged banks) per unit with tensor_tensor_reduce,
    writing the unit row-min to its own mins column.

    loop_iters: if set, wraps the compute body in an on-device Fori loop
    (with semaphore clears + engine barriers between iterations) for
    steady-state benchmarking. Results are identical."""
    import concourse.bass as bass
    from concourse import mybir
    from contextlib import ExitStack

    nc = bass.Bass("TRN2", target_bir_lowering=False, debug=False,
                   num_devices=NCORES)
    WR = nc.dram_tensor("WR", [KROWS, 2 * N], mybir.dt.float16,
                        kind="ExternalInput").ap()
    OUT = nc.dram_tensor("OUT", [128, NUNITS3], mybir.dt.float32,
                         kind="ExternalOutput").ap()

    ctx = ExitStack()
    with ctx:
        wr = ctx.enter_context(
            nc.sbuf_tensor("wr_sb", [KROWS, 2 * N], mybir.dt.float16)).ap()
        mins = ctx.enter_context(
            nc.sbuf_tensor("mins_sb", [128, NUNITS3], mybir.dt.float32)).ap()
        scratch = [
            ctx.enter_context(
                nc.sbuf_tensor(f"scr{i}", [128, 1024], mybir.dt.float32)).ap()
            for i in range(2)
        ]
        junk = [
            ctx.enter_context(
                nc.sbuf_tensor(f"junk{i}", [128, 1024], mybir.dt.float32)).ap()
            for i in range(2)
        ]
        psum = ctx.enter_context(
            nc.psum_tensor("psum", [128, N], mybir.dt.float32)).ap()
        dma_sem = ctx.enter_context(nc.semaphore("dma_sem"))
        pe_sem = ctx.enter_context(nc.semaphore("pe_sem"))
        act_sem = ctx.enter_context(nc.semaphore("act_sem"))
        vec_sem = ctx.enter_context(nc.semaphore("vec_sem"))

        def w_ap(t):
            return wr[0:11, t * 128:(t + 1) * 128]

        def r_ap(c):
            return wr[0:11, N + c * 512:N + (c + 1) * 512]

        nc.sync.dma_start(out=wr[:], in_=WR[:]).then_inc(dma_sem, 16)

        def emit_extract(uu):
            # ScalarE: scan uu's tail (last column of junk[uu%2]) -> mins.
            # Emitted right after the stage of unit uu+2, whose vec_sem wait
            # (>= uu+1) is exactly this op's dependency -- it never adds a
            # stall to the Act queue.
            nc.scalar.copy(mins[:, uu:uu + 1], junk[uu % 2][:, 1023:1024]
                           ).then_inc(act_sem, 1)

        def emit_body():
            nc.tensor.wait_ge(dma_sem, 16)
            n_act = 0
            for u in range(NUNITS3):
                t, h = u // 2, u % 2
                base = 2048 * h
                # Staged pair (banks 2,3) first: they only need the STAGE of
                # unit u-2 to have drained them (act), not its scan -- this
                # keeps the PE and ScalarE off each other's critical cycle.
                if u >= 2:
                    nc.tensor.wait_ge(act_sem,
                                      2 * u - 5 if u >= 4 else u - 1)
                for j in (2, 3):
                    mm = nc.tensor.matmul(
                        psum[:, base + 512 * j:base + 512 * (j + 1)],
                        w_ap(t), r_ap(4 * h + j), start=True, stop=True)
                mm.then_inc(pe_sem, 1)
                # PSUM pair (banks 0,1): consumed by scan u-2.
                if u >= 2:
                    nc.tensor.wait_ge(vec_sem, u - 1)
                for j in (0, 1):
                    mm = nc.tensor.matmul(
                        psum[:, base + 512 * j:base + 512 * (j + 1)],
                        w_ap(t), r_ap(4 * h + j), start=True, stop=True)
                mm.then_inc(pe_sem, 1)

                # ScalarE: banks {2,3} of the quad -> SBUF scratch
                nc.scalar.wait_ge(pe_sem, 2 * u + 1)
                if u >= 2:
                    nc.scalar.wait_ge(vec_sem, u - 1)  # scratch WAR vs scan
                nc.scalar.copy(scratch[h][:, :],
                               psum[:, base + 1024:base + 2048]
                               ).then_inc(act_sem, 1)
                n_act += 1
                if u >= 2:
                    emit_extract(u - 2)
                    n_act += 1

                # VectorE: running min over (psum-pair min staged-pair); the
                # unit's row-min lands in the last scan column of junk[h].
                # The act wait covers this unit's stage AND the extract of
                # unit u-2 (junk[h] WAR).
                nc.vector.wait_ge(pe_sem, 2 * u + 2)
                nc.vector.wait_ge(act_sem, n_act)
                nc.vector.tensor_tensor_scan(
                    out=junk[h][:, :],
                    data0=psum[:, base:base + 1024],
                    data1=scratch[h][:, :], initial=FMAX,
                    op0=mybir.AluOpType.min, op1=mybir.AluOpType.min,
                ).then_inc(vec_sem, 1)

            for uu in (NUNITS3 - 2, NUNITS3 - 1):
                nc.scalar.wait_ge(vec_sem, uu + 1)
                emit_extract(uu)
                n_act += 1
            return n_act

        if loop_iters is None:
            total_act = emit_body()
            nc.sync.wait_ge(act_sem, total_act)
        else:
            with nc.Fori(0, loop_iters):
                emit_body()
                nc.all_engine_barrier()
                nc.vector.sem_clear(pe_sem)
                nc.vector.sem_clear(act_sem)
                nc.vector.sem_clear(vec_sem)
                nc.all_engine_barrier()

        nc.sync.dma_start(out=OUT[:], in_=mins[:]).then_inc(dma_sem, 16)
        nc.sync.wait_ge(dma_sem, 32)

    return nc


def _pack_core_inputs_v3(P: np.ndarray, S: np.ndarray):
    """P: [N, 3] query points, S: [N, 3] database points.

    Returns WR fp16 [KROWS, 2N]: cols [:N] = stationary W, cols [N:] = R.
      W rows: [Ph.T(3), Ph.T(3), Pl.T(3), 1, 1, pad]
      R rows: [Uh.T(3), Ul.T(3), Uh.T(3), s2h, s2l, pad]  (U = -2 S)
    """
    f16 = np.float16
    P = P.astype(np.float32)
    S = S.astype(np.float32)

    Ph = P.astype(f16)
    Pl = (P - Ph.astype(np.float32)).astype(f16)
    U = -2.0 * S
    Uh = U.astype(f16)
    Ul = (U - Uh.astype(np.float32)).astype(f16)
    s2 = (S ** 2).sum(-1)
    s2h = s2.astype(f16)
    s2l = (s2 - s2h.astype(np.float32)).astype(f16)

    W = np.zeros((KROWS, N), f16)
    W[0:3] = Ph.T
    W[3:6] = Ph.T
    W[6:9] = Pl.T
    W[9] = 1.0
    W[10] = 1.0
    R = np.zeros((KROWS, N), f16)
    R[0:3] = Uh.T
    R[3:6] = Ul.T
    R[6:9] = Uh.T
    R[9] = s2h
    R[10] = s2l
    return np.ascontiguousarray(np.concatenate([W, R], axis=1))


def _unpack_mins(mins: np.ndarray) -> np.ndarray:
    """-> per-query min over db of (-2 q.s + ||s||^2), indexed by query n."""
    cols = 4 if VERSION in (4, 5) else 2
    m = mins.reshape(128, NTILES, cols).min(axis=2)  # [p, t]
    return m.T.reshape(N)  # n = t*128 + p


def make_in_maps(set1: np.ndarray, set2: np.ndarray):
    """Per-core input maps + per-core query norms."""
    in_maps, qnorms = [], []
    for c in range(NCORES):
        b, ori = c // 2, c % 2
        P = set1[b] if ori == 0 else set2[b]
        S = set2[b] if ori == 0 else set1[b]
        WR = _pack_core_inputs_v3(P, S)
        in_maps.append({"WR": WR})
        qnorms.append((P.astype(np.float32) ** 2).sum(-1))
    return in_maps, qnorms


def _get_nc():
    global _nc_cache
    if _nc_cache is None:
        _nc_cache = {3: _build_nc_v3, 4: _build_nc_v4,
                     5: _build_nc_v5}[VERSION]()
    return _nc_cache


def kernel(set1: np.ndarray, set2: np.ndarray) -> np.ndarray:
    from concourse.bass_utils import run_bass_kernel_spmd

    set1 = np.asarray(set1, dtype=np.float32)
    set2 = np.asarray(set2, dtype=np.float32)

    nc = _get_nc()
    in_maps, qnorms = make_in_maps(set1, set2)
    res = run_bass_kernel_spmd(nc, in_maps, list(range(NCORES)))
    terms = []
    for c in range(NCORES):
        raw = _unpack_mins(np.asarray(res.results[c]["OUT"]))
        d2 = np.maximum(raw + qnorms[c], 0.0).astype(np.float32)
        terms.append(np.sqrt(d2).mean(dtype=np.float32))
    total = np.mean([terms[2 * b] + terms[2 * b + 1] for b in range(B)],
                    dtype=np.float32)
    return np.array(total, dtype=np.float32)


# revision 22
# speedup vs baseline: 1.0385x; 1.0385x over previous
"""Averaged Hausdorff loss on 8 Trainium2 NeuronCores.

Problem: set1, set2 [B=4, N=4096, D=3] fp32.
  dist[b, n, m] = ||set1[b,n] - set2[b,m]||
  out = mean_b( mean_n min_m dist + mean_m min_n dist )

Sharding: one core per (batch, orientation) pair -> exactly 8 cores.
  core 2b+0: row mins  (queries = set1[b], database = set2[b])
  core 2b+1: col mins  (queries = set2[b], database = set1[b])

Kernel (per core): with q = query point, s = database point,
  d2(q, s) = ||q||^2 + (||s||^2 - 2 q.s)
The parenthesized part is ONE K=11 fp16 matmul: matmul cost on the PE is
(moving columns) x (cycles/column) regardless of K, so the three hi/lo
precision passes of the old kernel (xh.uh + xh.ul + xl.uh, each K=4) stack
vertically into a single K=11 instruction at one third the PE time:
  lhsT rows = [qh, qh, ql, 1, 1]          (fp16 hi/lo split of q, 3+3+3+1+1)
  rhs  rows = [uh, ul, uh, s2h, s2l]      (u = -2 s, s2 = ||s||^2)
fp16 x fp16 products are exact in fp32 and the dropped xl.ul term is
~2^-22 relative, so the distance matrix is fp32-grade (measured final
rel err ~5e-5).

Matmuls rotate across the four 32-row PE-array strips via tile_position
(inputs replicated at partition groups 0/32/64/96); strip-tiled matmuls
overlap in the array on real hw: measured 72 ns vs 441 ns per 512-col
matmul (the cost model does not capture this).

Reduction (hw-measured op costs, which diverge badly from the cost model):
the DVE scan recurrence runs at ~2 cycles/element and superlinearly in free
size, so each (query-tile, half-db) unit issues two independent 512-free
tensor_tensor_scan(min,min) ops -- data0 a PSUM bank, data1 a ScalarE-staged
bank -- consuming the unit's 4 banks at the best measured rate
(~0.87 ns/element); ScalarE extracts both scan tails per unit with one
strided copy. VectorE is the bottleneck engine at ~114 us/core of scans;
ScalarE staging (~0.96 ns/elem) and the strip-parallel PE (~18 us) hide
under it.
"""

import os
import sys

import numpy as np

for _p in ("/opt/trn_rl_repo",):
    if _p not in sys.path and os.path.isdir(_p):
        sys.path.insert(0, _p)

B, N, D = 4, 4096, 3
NCORES = 8
NTILES = N // 128          # 32 query tiles of 128
NCHUNKS = N // 512         # 8 database chunks of 512
KROWS = 16                 # 11 used contraction rows, padded to 16
VERSION = 4
NUNITS3 = NTILES * 2       # 64 (tile, half-db) units
NUNITS5 = NTILES * 4       # 128 (tile, quarter-db) units
FMAX = 3.0e38

_nc_cache = None


def _build_nc_v5(loop_iters: int | None = None):
    """v5 (NOT used -- measured 147.9us vs v4's 143.4us; the finer units
    add per-scan overhead that outweighs the deeper rotation):
    2-bank units with 4-deep PSUM rotation.

    Each unit is one (query-tile, db-quarter): two strip-rotated matmuls
    (one bank ScalarE-staged, one scanned directly from PSUM), one 512-free
    VectorE scan. With 4 units in flight (8 banks), every cross-engine
    semaphore wait is satisfied several periods in advance, so the measured
    ~0.3-0.7us semaphore propagation latencies hide completely and the DVE
    runs back-to-back at its measured 893.5 ns/scan floor. The scan tails
    land in an 8-slot junk ring; ScalarE extracts pairs of tails with one
    strided copy each, ordered so extracts always trail the scans they read
    by several units.
    """
    import concourse.bass as bass
    from concourse import mybir
    from contextlib import ExitStack

    nc = bass.Bass("TRN2", target_bir_lowering=False, debug=False,
                   num_devices=NCORES)
    WR = nc.dram_tensor("WR", [KROWS, 2 * N], mybir.dt.float16,
                        kind="ExternalInput").ap()
    OUT = nc.dram_tensor("OUT", [128, NUNITS5], mybir.dt.float32,
                         kind="ExternalOutput").ap()

    ctx = ExitStack()
    with ctx:
        wr = ctx.enter_context(
            nc.sbuf_tensor("wr_sb", [128, 2 * N], mybir.dt.float16)).ap()
        mins = ctx.enter_context(
            nc.sbuf_tensor("mins_sb", [128, NUNITS5], mybir.dt.float32)).ap()
        scratch = ctx.enter_context(
            nc.sbuf_tensor("scr_sb", [128, 4, 512], mybir.dt.float32)).ap()
        junk = ctx.enter_context(
            nc.sbuf_tensor("junk_sb", [128, 8, 512], mybir.dt.float32)).ap()
        psum = ctx.enter_context(
            nc.psum_tensor("psum", [128, N], mybir.dt.float32)).ap()
        dma_sem = ctx.enter_context(nc.semaphore("dma_sem"))
        pe_sem = ctx.enter_context(nc.semaphore("pe_sem"))
        act_sem = ctx.enter_context(nc.semaphore("act_sem"))
        vec_sem = ctx.enter_context(nc.semaphore("vec_sem"))

        def w_ap(s, t):
            return wr[32 * s:32 * s + 11, t * 128:(t + 1) * 128]

        def r_ap(s, c):
            return wr[32 * s:32 * s + 11, N + c * 512:N + (c + 1) * 512]

        for s in range(4):
            nc.sync.dma_start(out=wr[32 * s:32 * s + KROWS, :],
                              in_=WR[:]).then_inc(dma_sem, 16)

        def emit_body():
            nc.tensor.wait_ge(dma_sem, 64)
            stage_idx = {}
            n_act = 0
            n_extract = 0

            def emit_extract(k):
                # tails of units 2k, 2k+1 (junk slots (2k)%8, (2k)%8+1)
                j0 = (2 * k) % 8
                nc.scalar.copy(mins[:, 2 * k:2 * k + 2],
                               junk[:, j0:j0 + 2, 511]
                               ).then_inc(act_sem, 1)

            for u in range(NUNITS5):
                t, q = u // 4, u % 4
                slot = u % 4
                bank_s = 2 * slot + 1
                bank_p = 2 * slot
                # staged-bank matmul (even strip): its bank was drained by
                # stage(u-4).
                if u >= 4:
                    nc.tensor.wait_ge(act_sem, stage_idx[u - 4])
                mm = nc.tensor.matmul(
                    psum[:, 512 * bank_s:512 * (bank_s + 1)],
                    w_ap((2 * u) % 4, t), r_ap((2 * u) % 4, 2 * q + 1),
                    start=True, stop=True,
                    tile_position=(32 * ((2 * u) % 4), 0))
                mm.then_inc(pe_sem, 1)
                # scan-bank matmul (odd strip): its bank was read by scan(u-4)
                if u >= 4:
                    nc.tensor.wait_ge(vec_sem, u - 3)
                mm = nc.tensor.matmul(
                    psum[:, 512 * bank_p:512 * (bank_p + 1)],
                    w_ap((2 * u + 1) % 4, t), r_ap((2 * u + 1) % 4, 2 * q),
                    start=True, stop=True,
                    tile_position=(32 * ((2 * u + 1) % 4), 0))
                mm.then_inc(pe_sem, 1)

                # ScalarE: staged bank -> scratch slot
                nc.scalar.wait_ge(pe_sem, 2 * u + 1)
                if u >= 4:
                    nc.scalar.wait_ge(vec_sem, u - 3)  # scratch WAR
                nc.scalar.copy(scratch[:, slot, :],
                               psum[:, 512 * bank_s:512 * (bank_s + 1)]
                               ).then_inc(act_sem, 1)
                n_act += 1
                stage_idx[u] = n_act
                # Extract tails for pair k once scan(2k+1) is implied by this
                # stage's own vec wait (u - 3 >= 2k + 2  =>  k = (u-5)//2).
                if u >= 5 and u % 2 == 1:
                    emit_extract((u - 5) // 2)
                    n_act += 1
                    n_extract += 1

                # VectorE: one 512-free running-min scan; junk-ring WAR vs
                # the extract that read slot u%8 (emitted after stage(u-3)).
                nc.vector.wait_ge(pe_sem, 2 * u + 2)
                nc.vector.wait_ge(act_sem, stage_idx[u])
                nc.vector.tensor_tensor_scan(
                    out=junk[:, u % 8, :],
                    data0=psum[:, 512 * bank_p:512 * (bank_p + 1)],
                    data1=scratch[:, slot, :],
                    initial=FMAX,
                    op0=mybir.AluOpType.min, op1=mybir.AluOpType.min,
                ).then_inc(vec_sem, 1)

            while n_extract < NUNITS5 // 2:
                k = n_extract
                nc.scalar.wait_ge(vec_sem, 2 * k + 2)
                emit_extract(k)
                n_act += 1
                n_extract += 1
            return n_act

        if loop_iters is None:
            total_act = emit_body()
            nc.sync.wait_ge(act_sem, total_act)
        else:
            with nc.Fori(0, loop_iters):
                emit_body()
                nc.all_engine_barrier()
                nc.vector.sem_clear(pe_sem)
                nc.vector.sem_clear(act_sem)
                nc.vector.sem_clear(vec_sem)
                nc.all_engine_barrier()

        nc.sync.dma_start(out=OUT[:], in_=mins[:]).then_inc(dma_sem, 16)
        nc.sync.wait_ge(dma_sem, 80)

    return nc


def _build_nc_v4(loop_iters: int | None = None):
    """v4: like v3 but with the two hw-measured wins the cost model misses:

    1. Matmuls rotate across the four 32-row PE-array strips via
       tile_position=(32s, 0) (inputs replicated at partition groups
       0/32/64/96): strip-tiled matmuls overlap in the array, measured
       72 ns vs 441 ns per 512-col matmul.
    2. The DVE scan recurrence runs at ~2 cycles/element and its cost is
       superlinear in free size, so each unit issues two independent
       512-free scans (FMAX initial; chaining through `initial` costs
       +350 ns/op) and ScalarE extracts both tails with one strided copy.
    """
    import concourse.bass as bass
    from concourse import mybir
    from contextlib import ExitStack

    nc = bass.Bass("TRN2", target_bir_lowering=False, debug=False,
                   num_devices=NCORES)
    WR = nc.dram_tensor("WR", [KROWS, 2 * N], mybir.dt.float16,
                        kind="ExternalInput").ap()
    OUT = nc.dram_tensor("OUT", [128, 2 * NUNITS3], mybir.dt.float32,
                         kind="ExternalOutput").ap()

    ctx = ExitStack()
    with ctx:
        wr = ctx.enter_context(
            nc.sbuf_tensor("wr_sb", [128, 2 * N], mybir.dt.float16)).ap()
        mins = ctx.enter_context(
            nc.sbuf_tensor("mins_sb", [128, 2 * NUNITS3],
                           mybir.dt.float32)).ap()
        scratch = [
            ctx.enter_context(
                nc.sbuf_tensor(f"scr{i}", [128, 1024], mybir.dt.float32)).ap()
            for i in range(2)
        ]
        junk = [
            ctx.enter_context(
                nc.sbuf_tensor(f"junk{i}", [128, 1024], mybir.dt.float32)).ap()
            for i in range(2)
        ]
        psum = ctx.enter_context(
            nc.psum_tensor("psum", [128, N], mybir.dt.float32)).ap()
        dma_sem = ctx.enter_context(nc.semaphore("dma_sem"))
        pe_sem = ctx.enter_context(nc.semaphore("pe_sem"))
        act_sem = ctx.enter_context(nc.semaphore("act_sem"))
        vec_sem = ctx.enter_context(nc.semaphore("vec_sem"))

        def w_ap(s, t):
            return wr[32 * s:32 * s + 11, t * 128:(t + 1) * 128]

        def r_ap(s, c):
            return wr[32 * s:32 * s + 11, N + c * 512:N + (c + 1) * 512]

        for s in range(4):
            nc.sync.dma_start(out=wr[32 * s:32 * s + KROWS, :],
                              in_=WR[:]).then_inc(dma_sem, 16)

        def stage_idx(uu):
            # act-queue op index of stage(uu): units 0,1 emit only a stage;
            # units >= 2 emit [stage(u), extract(u-2)].
            return uu + 1 if uu < 2 else 2 * uu - 1

        def emit_extract(uu):
            # Both scan tails of unit uu (cols 511, 1023 of junk[uu%2])
            # -> mins cols 2uu, 2uu+1, one strided ScalarE copy.
            j = junk[uu % 2]
            nc.scalar.copy(mins[:, 2 * uu:2 * uu + 2],
                           j.rearrange("p (two f) -> p two f", two=2)[:, :, 511]
                           ).then_inc(act_sem, 1)

        def emit_body():
            nc.tensor.wait_ge(dma_sem, 64)
            n_act = 0
            for u in range(NUNITS3):
                t, h = u // 2, u % 2
                base = 2048 * h
                # Staged pair (banks 2,3 / strips 2,3) first: they only
                # need stage(u-2) to have drained them.
                if u >= 2:
                    nc.tensor.wait_ge(act_sem, stage_idx(u - 2))
                for j in (2, 3):
                    mm = nc.tensor.matmul(
                        psum[:, base + 512 * j:base + 512 * (j + 1)],
                        w_ap(j, t), r_ap(j, 4 * h + j), start=True, stop=True,
                        tile_position=(32 * j, 0))
                mm.then_inc(pe_sem, 1)
                # Scan pair (banks 0,1 / strips 0,1): consumed by scans u-2.
                if u >= 2:
                    nc.tensor.wait_ge(vec_sem, 2 * u - 2)
                for j in (0, 1):
                    mm = nc.tensor.matmul(
                        psum[:, base + 512 * j:base + 512 * (j + 1)],
                        w_ap(j, t), r_ap(j, 4 * h + j), start=True, stop=True,
                        tile_position=(32 * j, 0))
                mm.then_inc(pe_sem, 1)

                # ScalarE: banks {2,3} of the quad -> SBUF scratch
                nc.scalar.wait_ge(pe_sem, 2 * u + 1)
                if u >= 2:
                    nc.scalar.wait_ge(vec_sem, 2 * u - 2)  # scratch WAR
                nc.scalar.copy(scratch[h][:, :],
                               psum[:, base + 1024:base + 2048]
                               ).then_inc(act_sem, 1)
                n_act += 1
                if u >= 2:
                    emit_extract(u - 2)
                    n_act += 1

                # VectorE: two independent 512-free running-min scans; the
                # act wait covers this unit's stage AND the extract of unit
                # u-2 (junk[h] WAR).
                nc.vector.wait_ge(pe_sem, 2 * u + 2)
                nc.vector.wait_ge(act_sem, n_act)
                for k in range(2):
                    nc.vector.tensor_tensor_scan(
                        out=junk[h][:, 512 * k:512 * (k + 1)],
                        data0=psum[:, base + 512 * k:base + 512 * (k + 1)],
                        data1=scratch[h][:, 512 * k:512 * (k + 1)],
                        initial=FMAX,
                        op0=mybir.AluOpType.min, op1=mybir.AluOpType.min,
                    ).then_inc(vec_sem, 1)

            for uu in (NUNITS3 - 2, NUNITS3 - 1):
                nc.scalar.wait_ge(vec_sem, 2 * uu + 2)
                emit_extract(uu)
                n_act += 1
            return n_act

        if loop_iters is None:
            total_act = emit_body()
            nc.sync.wait_ge(act_sem, total_act)
        else:
            with nc.Fori(0, loop_iters):
                emit_body()
                nc.all_engine_barrier()
                nc.vector.sem_clear(pe_sem)
                nc.vector.sem_clear(act_sem)
                nc.vector.sem_clear(vec_sem)
                nc.all_engine_barrier()

        nc.sync.dma_start(out=OUT[:], in_=mins[:]).then_inc(dma_sem, 16)
        nc.sync.wait_ge(dma_sem, 80)

    return nc


def _build_nc_v3(loop_iters: int | None = None):
    """v3 (fallback, measured 145.8us): PE streams one K=11 fp16 matmul per
    (tile, chunk) into a rotating half of PSUM (no strip rotation); ScalarE
    stages the odd bank-pair of each 4-bank unit to SBUF; VectorE runs one
    1024-free tensor_tensor_scan(min,min) over (2 PSUM banks, 2 staged
    banks) per unit; ScalarE extracts the scan tail.

    loop_iters: if set, wraps the compute body in an on-device Fori loop
    (with semaphore clears + engine barriers between iterations) for
    steady-state benchmarking. Results are identical."""
    import concourse.bass as bass
    from concourse import mybir
    from contextlib import ExitStack

    nc = bass.Bass("TRN2", target_bir_lowering=False, debug=False,
                   num_devices=NCORES)
    WR = nc.dram_tensor("WR", [KROWS, 2 * N], mybir.dt.float16,
                        kind="ExternalInput").ap()
    OUT = nc.dram_tensor("OUT", [128, NUNITS3], mybir.dt.float32,
                         kind="ExternalOutput").ap()

    ctx = ExitStack()
    with ctx:
        wr = ctx.enter_context(
            nc.sbuf_tensor("wr_sb", [KROWS, 2 * N], mybir.dt.float16)).ap()
        mins = ctx.enter_context(
            nc.sbuf_tensor("mins_sb", [128, NUNITS3], mybir.dt.float32)).ap()
        scratch = [
            ctx.enter_context(
                nc.sbuf_tensor(f"scr{i}", [128, 1024], mybir.dt.float32)).ap()
            for i in range(2)
        ]
        junk = [
            ctx.enter_context(
                nc.sbuf_tensor(f"junk{i}", [128, 1024], mybir.dt.float32)).ap()
            for i in range(2)
        ]
        psum = ctx.enter_context(
            nc.psum_tensor("psum", [128, N], mybir.dt.float32)).ap()
        dma_sem = ctx.enter_context(nc.semaphore("dma_sem"))
        pe_sem = ctx.enter_context(nc.semaphore("pe_sem"))
        act_sem = ctx.enter_context(nc.semaphore("act_sem"))
        vec_sem = ctx.enter_context(nc.semaphore("vec_sem"))

        def w_ap(t):
            return wr[0:11, t * 128:(t + 1) * 128]

        def r_ap(c):
            return wr[0:11, N + c * 512:N + (c + 1) * 512]

        nc.sync.dma_start(out=wr[:], in_=WR[:]).then_inc(dma_sem, 16)

        def emit_extract(uu):
            # ScalarE: scan uu's tail (last column of junk[uu%2]) -> mins.
            # Emitted right after the stage of unit uu+2, whose vec_sem wait
            # (>= uu+1) is exactly this op's dependency -- it never adds a
            # stall to the Act queue.
            nc.scalar.copy(mins[:, uu:uu + 1], junk[uu % 2][:, 1023:1024]
                           ).then_inc(act_sem, 1)

        def emit_body():
            nc.tensor.wait_ge(dma_sem, 16)
            n_act = 0
            for u in range(NUNITS3):
                t, h = u // 2, u % 2
                base = 2048 * h
                # Staged pair (banks 2,3) first: they only need the STAGE of
                # unit u-2 to have drained them (act), not its scan -- this
                # keeps the PE and ScalarE off each other's critical cycle.
                if u >= 2:
                    nc.tensor.wait_ge(act_sem,
                                      2 * u - 5 if u >= 4 else u - 1)
                for j in (2, 3):
                    mm = nc.tensor.matmul(
                        psum[:, base + 512 * j:base + 512 * (j + 1)],
                        w_ap(t), r_ap(4 * h + j), start=True, stop=True)
                mm.then_inc(pe_sem, 1)
                # PSUM pair (banks 0,1): consumed by scan u-2.
                if u >= 2:
                    nc.tensor.wait_ge(vec_sem, u - 1)
                for j in (0, 1):
                    mm = nc.tensor.matmul(
                        psum[:, base + 512 * j:base + 512 * (j + 1)],
                        w_ap(t), r_ap(4 * h + j), start=True, stop=True)
                mm.then_inc(pe_sem, 1)

                # ScalarE: banks {2,3} of the quad -> SBUF scratch
                nc.scalar.wait_ge(pe_sem, 2 * u + 1)
                if u >= 2:
                    nc.scalar.wait_ge(vec_sem, u - 1)  # scratch WAR vs scan
                nc.scalar.copy(scratch[h][:, :],
                               psum[:, base + 1024:base + 2048]
                               ).then_inc(act_sem, 1)
                n_act += 1
                if u >= 2:
                    emit_extract(u - 2)
                    n_act += 1

                # VectorE: running min over (psum-pair min staged-pair); the
                # unit's row-min lands in the last scan column of junk[h].
                # The act wait covers this unit's stage AND the extract of
                # unit u-2 (junk[h] WAR).
                nc.vector.wait_ge(pe_sem, 2 * u + 2)
                nc.vector.wait_ge(act_sem, n_act)
                nc.vector.tensor_tensor_scan(
                    out=junk[h][:, :],
                    data0=psum[:, base:base + 1024],
                    data1=scratch[h][:, :], initial=FMAX,
                    op0=mybir.AluOpType.min, op1=mybir.AluOpType.min,
                ).then_inc(vec_sem, 1)

            for uu in (NUNITS3 - 2, NUNITS3 - 1):
                nc.scalar.wait_ge(vec_sem, uu + 1)
                emit_extract(uu)
                n_act += 1
            return n_act

        if loop_iters is None:
            total_act = emit_body()
            nc.sync.wait_ge(act_sem, total_act)
        else:
            with nc.Fori(0, loop_iters):
                emit_body()
                nc.all_engine_barrier()
                nc.vector.sem_clear(pe_sem)
                nc.vector.sem_clear(act_sem)
                nc.vector.sem_clear(vec_sem)
                nc.all_engine_barrier()

        nc.sync.dma_start(out=OUT[:], in_=mins[:]).then_inc(dma_sem, 16)
        nc.sync.wait_ge(dma_sem, 32)

    return nc


def _pack_core_inputs_v3(P: np.ndarray, S: np.ndarray):
    """P: [N, 3] query points, S: [N, 3] database points.

    Returns WR fp16 [KROWS, 2N]: cols [:N] = stationary W, cols [N:] = R.
      W rows: [Ph.T(3), Ph.T(3), Pl.T(3), 1, 1, pad]
      R rows: [Uh.T(3), Ul.T(3), Uh.T(3), s2h, s2l, pad]  (U = -2 S)
    """
    f16 = np.float16
    P = P.astype(np.float32)
    S = S.astype(np.float32)

    Ph = P.astype(f16)
    Pl = (P - Ph.astype(np.float32)).astype(f16)
    U = -2.0 * S
    Uh = U.astype(f16)
    Ul = (U - Uh.astype(np.float32)).astype(f16)
    s2 = (S ** 2).sum(-1)
    s2h = s2.astype(f16)
    s2l = (s2 - s2h.astype(np.float32)).astype(f16)

    W = np.zeros((KROWS, N), f16)
    W[0:3] = Ph.T
    W[3:6] = Ph.T
    W[6:9] = Pl.T
    W[9] = 1.0
    W[10] = 1.0
    R = np.zeros((KROWS, N), f16)
    R[0:3] = Uh.T
    R[3:6] = Ul.T
    R[6:9] = Uh.T
    R[9] = s2h
    R[10] = s2l
    return np.ascontiguousarray(np.concatenate([W, R], axis=1))


def _unpack_mins(mins: np.ndarray) -> np.ndarray:
    """-> per-query min over db of (-2 q.s + ||s||^2), indexed by query n."""
    cols = 4 if VERSION in (4, 5) else 2
    m = mins.reshape(128, NTILES, cols).min(axis=2)  # [p, t]
    return m.T.reshape(N)  # n = t*128 + p


def make_in_maps(set1: np.ndarray, set2: np.ndarray):
    """Per-core input maps + per-core query norms."""
    in_maps, qnorms = [], []
    for c in range(NCORES):
        b, ori = c // 2, c % 2
        P = set1[b] if ori == 0 else set2[b]
        S = set2[b] if ori == 0 else set1[b]
        WR = _pack_core_inputs_v3(P, S)
        in_maps.append({"WR": WR})
        qnorms.append((P.astype(np.float32) ** 2).sum(-1))
    return in_maps, qnorms


def _get_nc():
    global _nc_cache
    if _nc_cache is None:
        _nc_cache = {3: _build_nc_v3, 4: _build_nc_v4,
                     5: _build_nc_v5}[VERSION]()
    return _nc_cache


def kernel(set1: np.ndarray, set2: np.ndarray) -> np.ndarray:
    from concourse.bass_utils import run_bass_kernel_spmd

    set1 = np.asarray(set1, dtype=np.float32)
    set2 = np.asarray(set2, dtype=np.float32)

    nc = _get_nc()
    in_maps, qnorms = make_in_maps(set1, set2)
    res = run_bass_kernel_spmd(nc, in_maps, list(range(NCORES)))
    terms = []
    for c in range(NCORES):
        raw = _unpack_mins(np.asarray(res.results[c]["OUT"]))
        d2 = np.maximum(raw + qnorms[c], 0.0).astype(np.float32)
        terms.append(np.sqrt(d2).mean(dtype=np.float32))
    total = np.mean([terms[2 * b] + terms[2 * b + 1] for b in range(B)],
                    dtype=np.float32)
    return np.array(total, dtype=np.float32)


# revision 25
# speedup vs baseline: 1.2267x; 1.1812x over previous
"""Averaged Hausdorff loss on 8 Trainium2 NeuronCores.

Problem: set1, set2 [B=4, N=4096, D=3] fp32.
  dist[b, n, m] = ||set1[b,n] - set2[b,m]||
  out = mean_b( mean_n min_m dist + mean_m min_n dist )

Sharding: one core per (batch, orientation) pair -> exactly 8 cores.
  core 2b+0: row mins  (queries = set1[b], database = set2[b])
  core 2b+1: col mins  (queries = set2[b], database = set1[b])

Kernel (per core): with q = query point, s = database point,
  d2(q, s) = ||q||^2 + (||s||^2 - 2 q.s)
The parenthesized part is ONE K=11 fp16 matmul: matmul cost on the PE is
(moving columns) x (cycles/column) regardless of K, so the three hi/lo
precision passes of the old kernel (xh.uh + xh.ul + xl.uh, each K=4) stack
vertically into a single K=11 instruction at one third the PE time:
  lhsT rows = [qh, qh, ql, 1, 1]          (fp16 hi/lo split of q, 3+3+3+1+1)
  rhs  rows = [uh, ul, uh, s2h, s2l]      (u = -2 s, s2 = ||s||^2)
fp16 x fp16 products are exact in fp32 and the dropped xl.ul term is
~2^-22 relative, so the distance matrix is fp32-grade (measured final
rel err ~5e-5).

Matmuls rotate across the four 32-row PE-array strips via tile_position
(inputs replicated at partition groups 0/32/64/96); strip-tiled matmuls
overlap in the array on real hw: measured 72 ns vs 441 ns per 512-col
matmul (the cost model does not capture this).

Reduction (hw-measured op costs, which diverge badly from the cost model):
the DVE scan recurrence runs at ~2 cycles/element and superlinearly in free
size, so each (query-tile, half-db) unit issues two independent 512-free
tensor_tensor_scan(min,min) ops -- data0 a PSUM bank, data1 a ScalarE-staged
bank -- consuming the unit's 4 banks at the best measured rate
(~0.87 ns/element); ScalarE extracts both scan tails per unit with one
strided copy. VectorE is the bottleneck engine at ~114 us/core of scans;
ScalarE staging (~0.96 ns/elem) and the strip-parallel PE (~18 us) hide
under it.
"""

import os
import sys

import numpy as np

for _p in ("/opt/trn_rl_repo",):
    if _p not in sys.path and os.path.isdir(_p):
        sys.path.insert(0, _p)

B, N, D = 4, 4096, 3
NCORES = 8
NTILES = N // 128          # 32 query tiles of 128
NCHUNKS = N // 512         # 8 database chunks of 512
KROWS = 16                 # 11 used contraction rows, padded to 16
VERSION = 4
NUNITS3 = NTILES * 2       # 64 (tile, half-db) units
NUNITS5 = NTILES * 4       # 128 (tile, quarter-db) units
FMAX = 3.0e38

_nc_cache = None


def _build_nc_v5(loop_iters: int | None = None):
    """v5 (NOT used -- measured 147.9us vs v4's 143.4us; the finer units
    add per-scan overhead that outweighs the deeper rotation):
    2-bank units with 4-deep PSUM rotation.

    Each unit is one (query-tile, db-quarter): two strip-rotated matmuls
    (one bank ScalarE-staged, one scanned directly from PSUM), one 512-free
    VectorE scan. With 4 units in flight (8 banks), every cross-engine
    semaphore wait is satisfied several periods in advance, so the measured
    ~0.3-0.7us semaphore propagation latencies hide completely and the DVE
    runs back-to-back at its measured 893.5 ns/scan floor. The scan tails
    land in an 8-slot junk ring; ScalarE extracts pairs of tails with one
    strided copy each, ordered so extracts always trail the scans they read
    by several units.
    """
    import concourse.bass as bass
    from concourse import mybir
    from contextlib import ExitStack

    nc = bass.Bass("TRN2", target_bir_lowering=False, debug=False,
                   num_devices=NCORES)
    WR = nc.dram_tensor("WR", [KROWS, 2 * N], mybir.dt.float16,
                        kind="ExternalInput").ap()
    OUT = nc.dram_tensor("OUT", [128, NUNITS5], mybir.dt.float32,
                         kind="ExternalOutput").ap()

    ctx = ExitStack()
    with ctx:
        wr = ctx.enter_context(
            nc.sbuf_tensor("wr_sb", [128, 2 * N], mybir.dt.float16)).ap()
        mins = ctx.enter_context(
            nc.sbuf_tensor("mins_sb", [128, NUNITS5], mybir.dt.float32)).ap()
        scratch = ctx.enter_context(
            nc.sbuf_tensor("scr_sb", [128, 4, 512], mybir.dt.float32)).ap()
        junk = ctx.enter_context(
            nc.sbuf_tensor("junk_sb", [128, 8, 512], mybir.dt.float32)).ap()
        psum = ctx.enter_context(
            nc.psum_tensor("psum", [128, N], mybir.dt.float32)).ap()
        dma_sem = ctx.enter_context(nc.semaphore("dma_sem"))
        pe_sem = ctx.enter_context(nc.semaphore("pe_sem"))
        act_sem = ctx.enter_context(nc.semaphore("act_sem"))
        vec_sem = ctx.enter_context(nc.semaphore("vec_sem"))

        def w_ap(s, t):
            return wr[32 * s:32 * s + 11, t * 128:(t + 1) * 128]

        def r_ap(s, c):
            return wr[32 * s:32 * s + 11, N + c * 512:N + (c + 1) * 512]

        for s in range(4):
            nc.sync.dma_start(out=wr[32 * s:32 * s + KROWS, :],
                              in_=WR[:]).then_inc(dma_sem, 16)

        def emit_body():
            nc.tensor.wait_ge(dma_sem, 64)
            stage_idx = {}
            n_act = 0
            n_extract = 0

            def emit_extract(k):
                # tails of units 2k, 2k+1 (junk slots (2k)%8, (2k)%8+1)
                j0 = (2 * k) % 8
                nc.scalar.copy(mins[:, 2 * k:2 * k + 2],
                               junk[:, j0:j0 + 2, 511]
                               ).then_inc(act_sem, 1)

            for u in range(NUNITS5):
                t, q = u // 4, u % 4
                slot = u % 4
                bank_s = 2 * slot + 1
                bank_p = 2 * slot
                # staged-bank matmul (even strip): its bank was drained by
                # stage(u-4).
                if u >= 4:
                    nc.tensor.wait_ge(act_sem, stage_idx[u - 4])
                mm = nc.tensor.matmul(
                    psum[:, 512 * bank_s:512 * (bank_s + 1)],
                    w_ap((2 * u) % 4, t), r_ap((2 * u) % 4, 2 * q + 1),
                    start=True, stop=True,
                    tile_position=(32 * ((2 * u) % 4), 0))
                mm.then_inc(pe_sem, 1)
                # scan-bank matmul (odd strip): its bank was read by scan(u-4)
                if u >= 4:
                    nc.tensor.wait_ge(vec_sem, u - 3)
                mm = nc.tensor.matmul(
                    psum[:, 512 * bank_p:512 * (bank_p + 1)],
                    w_ap((2 * u + 1) % 4, t), r_ap((2 * u + 1) % 4, 2 * q),
                    start=True, stop=True,
                    tile_position=(32 * ((2 * u + 1) % 4), 0))
                mm.then_inc(pe_sem, 1)

                # ScalarE: staged bank -> scratch slot
                nc.scalar.wait_ge(pe_sem, 2 * u + 1)
                if u >= 4:
                    nc.scalar.wait_ge(vec_sem, u - 3)  # scratch WAR
                nc.scalar.copy(scratch[:, slot, :],
                               psum[:, 512 * bank_s:512 * (bank_s + 1)]
                               ).then_inc(act_sem, 1)
                n_act += 1
                stage_idx[u] = n_act
                # Extract tails for pair k once scan(2k+1) is implied by this
                # stage's own vec wait (u - 3 >= 2k + 2  =>  k = (u-5)//2).
                if u >= 5 and u % 2 == 1:
                    emit_extract((u - 5) // 2)
                    n_act += 1
                    n_extract += 1

                # VectorE: one 512-free running-min scan; junk-ring WAR vs
                # the extract that read slot u%8 (emitted after stage(u-3)).
                nc.vector.wait_ge(pe_sem, 2 * u + 2)
                nc.vector.wait_ge(act_sem, stage_idx[u])
                nc.vector.tensor_tensor_scan(
                    out=junk[:, u % 8, :],
                    data0=psum[:, 512 * bank_p:512 * (bank_p + 1)],
                    data1=scratch[:, slot, :],
                    initial=FMAX,
                    op0=mybir.AluOpType.min, op1=mybir.AluOpType.min,
                ).then_inc(vec_sem, 1)

            while n_extract < NUNITS5 // 2:
                k = n_extract
                nc.scalar.wait_ge(vec_sem, 2 * k + 2)
                emit_extract(k)
                n_act += 1
                n_extract += 1
            return n_act

        if loop_iters is None:
            total_act = emit_body()
            nc.sync.wait_ge(act_sem, total_act)
        else:
            with nc.Fori(0, loop_iters):
                emit_body()
                nc.all_engine_barrier()
                nc.vector.sem_clear(pe_sem)
                nc.vector.sem_clear(act_sem)
                nc.vector.sem_clear(vec_sem)
                nc.all_engine_barrier()

        nc.sync.dma_start(out=OUT[:], in_=mins[:]).then_inc(dma_sem, 16)
        nc.sync.wait_ge(dma_sem, 80)

    return nc


def _build_nc_v4(loop_iters: int | None = None):
    """v4: like v3 but with the two hw-measured wins the cost model misses:

    1. Matmuls rotate across the four 32-row PE-array strips via
       tile_position=(32s, 0) (inputs replicated at partition groups
       0/32/64/96): strip-tiled matmuls overlap in the array, measured
       72 ns vs 441 ns per 512-col matmul.
    2. The DVE scan recurrence runs at ~2 cycles/element and its cost is
       superlinear in free size, so each unit issues two independent
       512-free scans (FMAX initial; chaining through `initial` costs
       +350 ns/op) and ScalarE extracts both tails with one strided copy.
    """
    import concourse.bass as bass
    from concourse import mybir
    from contextlib import ExitStack

    nc = bass.Bass("TRN2", target_bir_lowering=False, debug=False,
                   num_devices=NCORES)
    WR = nc.dram_tensor("WR", [KROWS, 2 * N], mybir.dt.float16,
                        kind="ExternalInput").ap()
    OUT = nc.dram_tensor("OUT", [128, 2 * NUNITS3], mybir.dt.float32,
                         kind="ExternalOutput").ap()

    ctx = ExitStack()
    with ctx:
        wr = ctx.enter_context(
            nc.sbuf_tensor("wr_sb", [128, 2 * N], mybir.dt.float16)).ap()
        mins = ctx.enter_context(
            nc.sbuf_tensor("mins_sb", [128, 2 * NUNITS3],
                           mybir.dt.float32)).ap()
        scratch = [
            ctx.enter_context(
                nc.sbuf_tensor(f"scr{i}", [128, 1024], mybir.dt.float32)).ap()
            for i in range(2)
        ]
        junk = [
            ctx.enter_context(
                nc.sbuf_tensor(f"junk{i}", [128, 1024], mybir.dt.float32)).ap()
            for i in range(2)
        ]
        psum = ctx.enter_context(
            nc.psum_tensor("psum", [128, N], mybir.dt.float32)).ap()
        dma_sem = ctx.enter_context(nc.semaphore("dma_sem"))
        pe_sem = ctx.enter_context(nc.semaphore("pe_sem"))
        act_sem = ctx.enter_context(nc.semaphore("act_sem"))
        vec_sem = ctx.enter_context(nc.semaphore("vec_sem"))

        def w_ap(s, t):
            return wr[32 * s:32 * s + 11, t * 128:(t + 1) * 128]

        def r_ap(s, c):
            return wr[32 * s:32 * s + 11, N + c * 512:N + (c + 1) * 512]

        for s in range(4):
            nc.sync.dma_start(out=wr[32 * s:32 * s + KROWS, :],
                              in_=WR[:]).then_inc(dma_sem, 16)

        def stage_idx(uu):
            # act-queue op index of stage(uu): units 0,1 emit only a stage;
            # units >= 2 emit [extract(u-2), stage(u)] -- extract FIRST, so
            # the scan's act threshold (= this index) is reached at stage
            # completion with no extract in between.
            return uu + 1 if uu < 2 else 2 * uu

        def emit_extract(uu):
            # Both scan tails of unit uu (cols 511, 1023 of junk[uu%2])
            # -> mins cols 2uu, 2uu+1, one strided ScalarE copy.
            j = junk[uu % 2]
            nc.scalar.copy(mins[:, 2 * uu:2 * uu + 2],
                           j.rearrange("p (two f) -> p two f", two=2)[:, :, 511]
                           ).then_inc(act_sem, 1)

        def emit_body():
            nc.tensor.wait_ge(dma_sem, 64)
            n_act = 0
            for u in range(NUNITS3):
                t, h = u // 2, u % 2
                base = 2048 * h
                # Staged pair (banks 2,3 / strips 2,3) first: they only
                # need stage(u-2) to have drained them.
                if u >= 2:
                    nc.tensor.wait_ge(act_sem, stage_idx(u - 2))
                for j in (2, 3):
                    mm = nc.tensor.matmul(
                        psum[:, base + 512 * j:base + 512 * (j + 1)],
                        w_ap(j, t), r_ap(j, 4 * h + j), start=True, stop=True,
                        tile_position=(32 * j, 0))
                mm.then_inc(pe_sem, 1)
                # Scan pair (banks 0,1 / strips 0,1): consumed by scans u-2.
                if u >= 2:
                    nc.tensor.wait_ge(vec_sem, 2 * u - 2)
                for j in (0, 1):
                    mm = nc.tensor.matmul(
                        psum[:, base + 512 * j:base + 512 * (j + 1)],
                        w_ap(j, t), r_ap(j, 4 * h + j), start=True, stop=True,
                        tile_position=(32 * j, 0))
                mm.then_inc(pe_sem, 1)

                # ScalarE: banks {2,3} of the quad -> SBUF scratch
                # extract(u-2) first: same vec gate as the stage's scratch
                # WAR, but this way the scan's act threshold lands exactly at
                # stage completion.
                if u >= 2:
                    nc.scalar.wait_ge(vec_sem, 2 * u - 2)
                    emit_extract(u - 2)
                    n_act += 1
                nc.scalar.wait_ge(pe_sem, 2 * u + 1)
                if u >= 2:
                    nc.scalar.wait_ge(vec_sem, 2 * u - 2)  # scratch WAR
                nc.scalar.copy(scratch[h][:, :],
                               psum[:, base + 1024:base + 2048]
                               ).then_inc(act_sem, 1)
                n_act += 1

                # VectorE: two independent 512-free running-min scans; the
                # act wait covers this unit's stage AND the extract of unit
                # u-2 (junk[h] WAR).
                nc.vector.wait_ge(pe_sem, 2 * u + 2)
                nc.vector.wait_ge(act_sem, n_act)
                for k in range(2):
                    nc.vector.tensor_tensor_scan(
                        out=junk[h][:, 512 * k:512 * (k + 1)],
                        data0=psum[:, base + 512 * k:base + 512 * (k + 1)],
                        data1=scratch[h][:, 512 * k:512 * (k + 1)],
                        initial=FMAX,
                        op0=mybir.AluOpType.min, op1=mybir.AluOpType.min,
                    ).then_inc(vec_sem, 1)

            for uu in (NUNITS3 - 2, NUNITS3 - 1):
                nc.scalar.wait_ge(vec_sem, 2 * uu + 2)
                emit_extract(uu)
                n_act += 1
            return n_act

        if loop_iters is None:
            total_act = emit_body()
            nc.sync.wait_ge(act_sem, total_act)
        else:
            with nc.Fori(0, loop_iters):
                emit_body()
                nc.all_engine_barrier()
                nc.vector.sem_clear(pe_sem)
                nc.vector.sem_clear(act_sem)
                nc.vector.sem_clear(vec_sem)
                nc.all_engine_barrier()

        nc.sync.dma_start(out=OUT[:], in_=mins[:]).then_inc(dma_sem, 16)
        nc.sync.wait_ge(dma_sem, 80)

    return nc


def _build_nc_v3(loop_iters: int | None = None):
    """v3 (fallback, measured 145.8us): PE streams one K=11 fp16 matmul per
    (tile, chunk) into a rotating half of PSUM (no strip rotation); ScalarE
    stages the odd bank-pair of each 4-bank unit to SBUF; VectorE runs one
    1024-free tensor_tensor_scan(min,min) over (2 PSUM banks, 2 staged
    banks) per unit; ScalarE extracts the scan tail.

    loop_iters: if set, wraps the compute body in an on-device Fori loop
    (with semaphore clears + engine barriers between iterations) for
    steady-state benchmarking. Results are identical."""
    import concourse.bass as bass
    from concourse import mybir
    from contextlib import ExitStack

    nc = bass.Bass("TRN2", target_bir_lowering=False, debug=False,
                   num_devices=NCORES)
    WR = nc.dram_tensor("WR", [KROWS, 2 * N], mybir.dt.float16,
                        kind="ExternalInput").ap()
    OUT = nc.dram_tensor("OUT", [128, NUNITS3], mybir.dt.float32,
                         kind="ExternalOutput").ap()

    ctx = ExitStack()
    with ctx:
        wr = ctx.enter_context(
            nc.sbuf_tensor("wr_sb", [KROWS, 2 * N], mybir.dt.float16)).ap()
        mins = ctx.enter_context(
            nc.sbuf_tensor("mins_sb", [128, NUNITS3], mybir.dt.float32)).ap()
        scratch = [
            ctx.enter_context(
                nc.sbuf_tensor(f"scr{i}", [128, 1024], mybir.dt.float32)).ap()
            for i in range(2)
        ]
        junk = [
            ctx.enter_context(
                nc.sbuf_tensor(f"junk{i}", [128, 1024], mybir.dt.float32)).ap()
            for i in range(2)
        ]
        psum = ctx.enter_context(
            nc.psum_tensor("psum", [128, N], mybir.dt.float32)).ap()
        dma_sem = ctx.enter_context(nc.semaphore("dma_sem"))
        pe_sem = ctx.enter_context(nc.semaphore("pe_sem"))
        act_sem = ctx.enter_context(nc.semaphore("act_sem"))
        vec_sem = ctx.enter_context(nc.semaphore("vec_sem"))

        def w_ap(t):
            return wr[0:11, t * 128:(t + 1) * 128]

        def r_ap(c):
            return wr[0:11, N + c * 512:N + (c + 1) * 512]

        nc.sync.dma_start(out=wr[:], in_=WR[:]).then_inc(dma_sem, 16)

        def emit_extract(uu):
            # ScalarE: scan uu's tail (last column of junk[uu%2]) -> mins.
            # Emitted right after the stage of unit uu+2, whose vec_sem wait
            # (>= uu+1) is exactly this op's dependency -- it never adds a
            # stall to the Act queue.
            nc.scalar.copy(mins[:, uu:uu + 1], junk[uu % 2][:, 1023:1024]
                           ).then_inc(act_sem, 1)

        def emit_body():
            nc.tensor.wait_ge(dma_sem, 16)
            n_act = 0
            for u in range(NUNITS3):
                t, h = u // 2, u % 2
                base = 2048 * h
                # Staged pair (banks 2,3) first: they only need the STAGE of
                # unit u-2 to have drained them (act), not its scan -- this
                # keeps the PE and ScalarE off each other's critical cycle.
                if u >= 2:
                    nc.tensor.wait_ge(act_sem,
                                      2 * u - 5 if u >= 4 else u - 1)
                for j in (2, 3):
                    mm = nc.tensor.matmul(
                        psum[:, base + 512 * j:base + 512 * (j + 1)],
                        w_ap(t), r_ap(4 * h + j), start=True, stop=True)
                mm.then_inc(pe_sem, 1)
                # PSUM pair (banks 0,1): consumed by scan u-2.
                if u >= 2:
                    nc.tensor.wait_ge(vec_sem, u - 1)
                for j in (0, 1):
                    mm = nc.tensor.matmul(
                        psum[:, base + 512 * j:base + 512 * (j + 1)],
                        w_ap(t), r_ap(4 * h + j), start=True, stop=True)
                mm.then_inc(pe_sem, 1)

                # ScalarE: banks {2,3} of the quad -> SBUF scratch
                nc.scalar.wait_ge(pe_sem, 2 * u + 1)
                if u >= 2:
                    nc.scalar.wait_ge(vec_sem, u - 1)  # scratch WAR vs scan
                nc.scalar.copy(scratch[h][:, :],
                               psum[:, base + 1024:base + 2048]
                               ).then_inc(act_sem, 1)
                n_act += 1
                if u >= 2:
                    emit_extract(u - 2)
                    n_act += 1

                # VectorE: running min over (psum-pair min staged-pair); the
                # unit's row-min lands in the last scan column of junk[h].
                # The act wait covers this unit's stage AND the extract of
                # unit u-2 (junk[h] WAR).
                nc.vector.wait_ge(pe_sem, 2 * u + 2)
                nc.vector.wait_ge(act_sem, n_act)
                nc.vector.tensor_tensor_scan(
                    out=junk[h][:, :],
                    data0=psum[:, base:base + 1024],
                    data1=scratch[h][:, :], initial=FMAX,
                    op0=mybir.AluOpType.min, op1=mybir.AluOpType.min,
                ).then_inc(vec_sem, 1)

            for uu in (NUNITS3 - 2, NUNITS3 - 1):
                nc.scalar.wait_ge(vec_sem, uu + 1)
                emit_extract(uu)
                n_act += 1
            return n_act

        if loop_iters is None:
            total_act = emit_body()
            nc.sync.wait_ge(act_sem, total_act)
        else:
            with nc.Fori(0, loop_iters):
                emit_body()
                nc.all_engine_barrier()
                nc.vector.sem_clear(pe_sem)
                nc.vector.sem_clear(act_sem)
                nc.vector.sem_clear(vec_sem)
                nc.all_engine_barrier()

        nc.sync.dma_start(out=OUT[:], in_=mins[:]).then_inc(dma_sem, 16)
        nc.sync.wait_ge(dma_sem, 32)

    return nc


def _pack_core_inputs_v3(P: np.ndarray, S: np.ndarray):
    """P: [N, 3] query points, S: [N, 3] database points.

    Returns WR fp16 [KROWS, 2N]: cols [:N] = stationary W, cols [N:] = R.
      W rows: [Ph.T(3), Ph.T(3), Pl.T(3), 1, 1, pad]
      R rows: [Uh.T(3), Ul.T(3), Uh.T(3), s2h, s2l, pad]  (U = -2 S)
    """
    f16 = np.float16
    P = P.astype(np.float32)
    S = S.astype(np.float32)

    Ph = P.astype(f16)
    Pl = (P - Ph.astype(np.float32)).astype(f16)
    U = -2.0 * S
    Uh = U.astype(f16)
    Ul = (U - Uh.astype(np.float32)).astype(f16)
    s2 = (S ** 2).sum(-1)
    s2h = s2.astype(f16)
    s2l = (s2 - s2h.astype(np.float32)).astype(f16)

    W = np.zeros((KROWS, N), f16)
    W[0:3] = Ph.T
    W[3:6] = Ph.T
    W[6:9] = Pl.T
    W[9] = 1.0
    W[10] = 1.0
    R = np.zeros((KROWS, N), f16)
    R[0:3] = Uh.T
    R[3:6] = Ul.T
    R[6:9] = Uh.T
    R[9] = s2h
    R[10] = s2l
    return np.ascontiguousarray(np.concatenate([W, R], axis=1))


def _unpack_mins(mins: np.ndarray) -> np.ndarray:
    """-> per-query min over db of (-2 q.s + ||s||^2), indexed by query n."""
    cols = 4 if VERSION in (4, 5) else 2
    m = mins.reshape(128, NTILES, cols).min(axis=2)  # [p, t]
    return m.T.reshape(N)  # n = t*128 + p


def make_in_maps(set1: np.ndarray, set2: np.ndarray):
    """Per-core input maps + per-core query norms."""
    in_maps, qnorms = [], []
    for c in range(NCORES):
        b, ori = c // 2, c % 2
        P = set1[b] if ori == 0 else set2[b]
        S = set2[b] if ori == 0 else set1[b]
        WR = _pack_core_inputs_v3(P, S)
        in_maps.append({"WR": WR})
        qnorms.append((P.astype(np.float32) ** 2).sum(-1))
    return in_maps, qnorms


def _get_nc():
    global _nc_cache
    if _nc_cache is None:
        _nc_cache = {3: _build_nc_v3, 4: _build_nc_v4,
                     5: _build_nc_v5}[VERSION]()
    return _nc_cache


def kernel(set1: np.ndarray, set2: np.ndarray) -> np.ndarray:
    from concourse.bass_utils import run_bass_kernel_spmd

    set1 = np.asarray(set1, dtype=np.float32)
    set2 = np.asarray(set2, dtype=np.float32)

    nc = _get_nc()
    in_maps, qnorms = make_in_maps(set1, set2)
    res = run_bass_kernel_spmd(nc, in_maps, list(range(NCORES)))
    terms = []
    for c in range(NCORES):
        raw = _unpack_mins(np.asarray(res.results[c]["OUT"]))
        d2 = np.maximum(raw + qnorms[c], 0.0).astype(np.float32)
        terms.append(np.sqrt(d2).mean(dtype=np.float32))
    total = np.mean([terms[2 * b] + terms[2 * b + 1] for b in range(B)],
                    dtype=np.float32)
    return np.array(total, dtype=np.float32)


# revision 26
# speedup vs baseline: 1.2947x; 1.0555x over previous
"""Averaged Hausdorff loss on 8 Trainium2 NeuronCores.

Problem: set1, set2 [B=4, N=4096, D=3] fp32.
  dist[b, n, m] = ||set1[b,n] - set2[b,m]||
  out = mean_b( mean_n min_m dist + mean_m min_n dist )

Sharding: one core per (batch, orientation) pair -> exactly 8 cores.
  core 2b+0: row mins  (queries = set1[b], database = set2[b])
  core 2b+1: col mins  (queries = set2[b], database = set1[b])

Kernel (per core): with q = query point, s = database point,
  d2(q, s) = ||q||^2 + (||s||^2 - 2 q.s)
The parenthesized part is ONE K=11 fp16 matmul: matmul cost on the PE is
(moving columns) x (cycles/column) regardless of K, so the three hi/lo
precision passes of the old kernel (xh.uh + xh.ul + xl.uh, each K=4) stack
vertically into a single K=11 instruction at one third the PE time:
  lhsT rows = [qh, qh, ql, 1, 1]          (fp16 hi/lo split of q, 3+3+3+1+1)
  rhs  rows = [uh, ul, uh, s2h, s2l]      (u = -2 s, s2 = ||s||^2)
fp16 x fp16 products are exact in fp32 and the dropped xl.ul term is
~2^-22 relative, so the distance matrix is fp32-grade (measured final
rel err ~5e-5).

Matmuls rotate across the four 32-row PE-array strips via tile_position
(inputs replicated at partition groups 0/32/64/96); strip-tiled matmuls
overlap in the array on real hw: measured 72 ns vs 441 ns per 512-col
matmul (the cost model does not capture this).

Reduction (hw-measured op costs, which diverge badly from the cost model):
the DVE scan recurrence runs at ~2 cycles/element and superlinearly in free
size, so each (query-tile, half-db) unit issues two independent 512-free
tensor_tensor_scan(min,min) ops -- data0 a PSUM bank, data1 a ScalarE-staged
bank -- consuming the unit's 4 banks at the best measured rate
(~0.87 ns/element); ScalarE extracts both scan tails per unit with one
strided copy. VectorE is the bottleneck engine at ~114 us/core of scans;
ScalarE staging (~0.96 ns/elem) and the strip-parallel PE (~18 us) hide
under it.
"""

import os
import sys

import numpy as np

for _p in ("/opt/trn_rl_repo",):
    if _p not in sys.path and os.path.isdir(_p):
        sys.path.insert(0, _p)

B, N, D = 4, 4096, 3
NCORES = 8
NTILES = N // 128          # 32 query tiles of 128
NCHUNKS = N // 512         # 8 database chunks of 512
KROWS = 16                 # 11 used contraction rows, padded to 16
VERSION = 4
NUNITS3 = NTILES * 2       # 64 (tile, half-db) units
NUNITS5 = NTILES * 4       # 128 (tile, quarter-db) units
FMAX = 3.0e38

_nc_cache = None


def _build_nc_v5(loop_iters: int | None = None):
    """v5 (NOT used -- measured 147.9us vs v4's 143.4us; the finer units
    add per-scan overhead that outweighs the deeper rotation):
    2-bank units with 4-deep PSUM rotation.

    Each unit is one (query-tile, db-quarter): two strip-rotated matmuls
    (one bank ScalarE-staged, one scanned directly from PSUM), one 512-free
    VectorE scan. With 4 units in flight (8 banks), every cross-engine
    semaphore wait is satisfied several periods in advance, so the measured
    ~0.3-0.7us semaphore propagation latencies hide completely and the DVE
    runs back-to-back at its measured 893.5 ns/scan floor. The scan tails
    land in an 8-slot junk ring; ScalarE extracts pairs of tails with one
    strided copy each, ordered so extracts always trail the scans they read
    by several units.
    """
    import concourse.bass as bass
    from concourse import mybir
    from contextlib import ExitStack

    nc = bass.Bass("TRN2", target_bir_lowering=False, debug=False,
                   num_devices=NCORES)
    WR = nc.dram_tensor("WR", [KROWS, 2 * N], mybir.dt.float16,
                        kind="ExternalInput").ap()
    OUT = nc.dram_tensor("OUT", [128, NUNITS5], mybir.dt.float32,
                         kind="ExternalOutput").ap()

    ctx = ExitStack()
    with ctx:
        wr = ctx.enter_context(
            nc.sbuf_tensor("wr_sb", [128, 2 * N], mybir.dt.float16)).ap()
        mins = ctx.enter_context(
            nc.sbuf_tensor("mins_sb", [128, NUNITS5], mybir.dt.float32)).ap()
        scratch = ctx.enter_context(
            nc.sbuf_tensor("scr_sb", [128, 4, 512], mybir.dt.float32)).ap()
        junk = ctx.enter_context(
            nc.sbuf_tensor("junk_sb", [128, 8, 512], mybir.dt.float32)).ap()
        psum = ctx.enter_context(
            nc.psum_tensor("psum", [128, N], mybir.dt.float32)).ap()
        dma_sem = ctx.enter_context(nc.semaphore("dma_sem"))
        pe_sem = ctx.enter_context(nc.semaphore("pe_sem"))
        act_sem = ctx.enter_context(nc.semaphore("act_sem"))
        vec_sem = ctx.enter_context(nc.semaphore("vec_sem"))

        def w_ap(s, t):
            return wr[32 * s:32 * s + 11, t * 128:(t + 1) * 128]

        def r_ap(s, c):
            return wr[32 * s:32 * s + 11, N + c * 512:N + (c + 1) * 512]

        for s in range(4):
            nc.sync.dma_start(out=wr[32 * s:32 * s + KROWS, :],
                              in_=WR[:]).then_inc(dma_sem, 16)

        def emit_body():
            nc.tensor.wait_ge(dma_sem, 64)
            stage_idx = {}
            n_act = 0
            n_extract = 0

            def emit_extract(k):
                # tails of units 2k, 2k+1 (junk slots (2k)%8, (2k)%8+1)
                j0 = (2 * k) % 8
                nc.scalar.copy(mins[:, 2 * k:2 * k + 2],
                               junk[:, j0:j0 + 2, 511]
                               ).then_inc(act_sem, 1)

            for u in range(NUNITS5):
                t, q = u // 4, u % 4
                slot = u % 4
                bank_s = 2 * slot + 1
                bank_p = 2 * slot
                # staged-bank matmul (even strip): its bank was drained by
                # stage(u-4).
                if u >= 4:
                    nc.tensor.wait_ge(act_sem, stage_idx[u - 4])
                mm = nc.tensor.matmul(
                    psum[:, 512 * bank_s:512 * (bank_s + 1)],
                    w_ap((2 * u) % 4, t), r_ap((2 * u) % 4, 2 * q + 1),
                    start=True, stop=True,
                    tile_position=(32 * ((2 * u) % 4), 0))
                mm.then_inc(pe_sem, 1)
                # scan-bank matmul (odd strip): its bank was read by scan(u-4)
                if u >= 4:
                    nc.tensor.wait_ge(vec_sem, u - 3)
                mm = nc.tensor.matmul(
                    psum[:, 512 * bank_p:512 * (bank_p + 1)],
                    w_ap((2 * u + 1) % 4, t), r_ap((2 * u + 1) % 4, 2 * q),
                    start=True, stop=True,
                    tile_position=(32 * ((2 * u + 1) % 4), 0))
                mm.then_inc(pe_sem, 1)

                # ScalarE: staged bank -> scratch slot
                nc.scalar.wait_ge(pe_sem, 2 * u + 1)
                if u >= 4:
                    nc.scalar.wait_ge(vec_sem, u - 3)  # scratch WAR
                nc.scalar.copy(scratch[:, slot, :],
                               psum[:, 512 * bank_s:512 * (bank_s + 1)]
                               ).then_inc(act_sem, 1)
                n_act += 1
                stage_idx[u] = n_act
                # Extract tails for pair k once scan(2k+1) is implied by this
                # stage's own vec wait (u - 3 >= 2k + 2  =>  k = (u-5)//2).
                if u >= 5 and u % 2 == 1:
                    emit_extract((u - 5) // 2)
                    n_act += 1
                    n_extract += 1

                # VectorE: one 512-free running-min scan; junk-ring WAR vs
                # the extract that read slot u%8 (emitted after stage(u-3)).
                nc.vector.wait_ge(pe_sem, 2 * u + 2)
                nc.vector.wait_ge(act_sem, stage_idx[u])
                nc.vector.tensor_tensor_scan(
                    out=junk[:, u % 8, :],
                    data0=psum[:, 512 * bank_p:512 * (bank_p + 1)],
                    data1=scratch[:, slot, :],
                    initial=FMAX,
                    op0=mybir.AluOpType.min, op1=mybir.AluOpType.min,
                ).then_inc(vec_sem, 1)

            while n_extract < NUNITS5 // 2:
                k = n_extract
                nc.scalar.wait_ge(vec_sem, 2 * k + 2)
                emit_extract(k)
                n_act += 1
                n_extract += 1
            return n_act

        if loop_iters is None:
            total_act = emit_body()
            nc.sync.wait_ge(act_sem, total_act)
        else:
            with nc.Fori(0, loop_iters):
                emit_body()
                nc.all_engine_barrier()
                nc.vector.sem_clear(pe_sem)
                nc.vector.sem_clear(act_sem)
                nc.vector.sem_clear(vec_sem)
                nc.all_engine_barrier()

        nc.sync.dma_start(out=OUT[:], in_=mins[:]).then_inc(dma_sem, 16)
        nc.sync.wait_ge(dma_sem, 80)

    return nc


def _build_nc_v4(loop_iters: int | None = None):
    """v4: like v3 but with the two hw-measured wins the cost model misses:

    1. Matmuls rotate across the four 32-row PE-array strips via
       tile_position=(32s, 0) (inputs replicated at partition groups
       0/32/64/96): strip-tiled matmuls overlap in the array, measured
       72 ns vs 441 ns per 512-col matmul.
    2. The DVE scan recurrence runs at ~2 cycles/element and its cost is
       superlinear in free size, so each unit issues two independent
       512-free scans (FMAX initial; chaining through `initial` costs
       +350 ns/op) and ScalarE extracts both tails with one strided copy.
    """
    import concourse.bass as bass
    from concourse import mybir
    from contextlib import ExitStack

    nc = bass.Bass("TRN2", target_bir_lowering=False, debug=False,
                   num_devices=NCORES)
    WR = nc.dram_tensor("WR", [KROWS, 2 * N], mybir.dt.float16,
                        kind="ExternalInput").ap()
    OUT = nc.dram_tensor("OUT", [128, NUNITS3], mybir.dt.float32,
                         kind="ExternalOutput").ap()

    ctx = ExitStack()
    with ctx:
        wr = ctx.enter_context(
            nc.sbuf_tensor("wr_sb", [128, 2 * N], mybir.dt.float16)).ap()
        mins = ctx.enter_context(
            nc.sbuf_tensor("mins_sb", [128, NUNITS3],
                           mybir.dt.float32)).ap()
        scratch = [
            ctx.enter_context(
                nc.sbuf_tensor(f"scr{i}", [128, 1024], mybir.dt.float32)).ap()
            for i in range(2)
        ]
        junk = [
            ctx.enter_context(
                nc.sbuf_tensor(f"junk{i}", [128, 1024], mybir.dt.float32)).ap()
            for i in range(2)
        ]
        psum = ctx.enter_context(
            nc.psum_tensor("psum", [128, N], mybir.dt.float32)).ap()
        dma_sem = ctx.enter_context(nc.semaphore("dma_sem"))
        pe_sem = ctx.enter_context(nc.semaphore("pe_sem"))
        act_sem = ctx.enter_context(nc.semaphore("act_sem"))
        vec_sem = ctx.enter_context(nc.semaphore("vec_sem"))

        def w_ap(s, t):
            return wr[32 * s:32 * s + 11, t * 128:(t + 1) * 128]

        def r_ap(s, c):
            return wr[32 * s:32 * s + 11, N + c * 512:N + (c + 1) * 512]

        for s in range(4):
            nc.sync.dma_start(out=wr[32 * s:32 * s + KROWS, :],
                              in_=WR[:]).then_inc(dma_sem, 16)

        def stage_idx(uu):
            # act-queue op index of stage(uu): units 0,1 emit only a stage;
            # units >= 2 emit [extract(u-2), stage(u)] -- extract FIRST, so
            # the scan's act threshold (= this index) is reached at stage
            # completion with no extract in between.
            return uu + 1 if uu < 2 else 2 * uu

        def emit_extract(uu):
            # The single scan tail of unit uu (col 1023 of junk[uu%2])
            # -> mins col uu.
            nc.scalar.copy(mins[:, uu:uu + 1],
                           junk[uu % 2][:, 1023:1024]).then_inc(act_sem, 1)

        def emit_body():
            nc.tensor.wait_ge(dma_sem, 64)
            n_act = 0
            for u in range(NUNITS3):
                t, h = u // 2, u % 2
                base = 2048 * h
                # Staged pair (banks 2,3 / strips 2,3) first: they only
                # need stage(u-2) to have drained them.
                if u >= 2:
                    nc.tensor.wait_ge(act_sem, stage_idx(u - 2))
                for j in (2, 3):
                    mm = nc.tensor.matmul(
                        psum[:, base + 512 * j:base + 512 * (j + 1)],
                        w_ap(j, t), r_ap(j, 4 * h + j), start=True, stop=True,
                        tile_position=(32 * j, 0))
                mm.then_inc(pe_sem, 1)
                # Scan pair (banks 0,1 / strips 0,1): consumed by scan u-2.
                if u >= 2:
                    nc.tensor.wait_ge(vec_sem, u - 1)
                for j in (0, 1):
                    mm = nc.tensor.matmul(
                        psum[:, base + 512 * j:base + 512 * (j + 1)],
                        w_ap(j, t), r_ap(j, 4 * h + j), start=True, stop=True,
                        tile_position=(32 * j, 0))
                mm.then_inc(pe_sem, 1)

                # ScalarE: banks {2,3} of the quad -> SBUF scratch
                # extract(u-2) first: same vec gate as the stage's scratch
                # WAR, but this way the scan's act threshold lands exactly at
                # stage completion.
                if u >= 2:
                    nc.scalar.wait_ge(vec_sem, u - 1)
                    emit_extract(u - 2)
                    n_act += 1
                nc.scalar.wait_ge(pe_sem, 2 * u + 1)
                if u >= 2:
                    nc.scalar.wait_ge(vec_sem, u - 1)  # scratch WAR
                nc.scalar.copy(scratch[h][:, :],
                               psum[:, base + 1024:base + 2048]
                               ).then_inc(act_sem, 1)
                n_act += 1

                # VectorE: ONE 1024-free running-min scan over (psum pair,
                # staged pair); per-op pipeline overhead and queue traffic
                # halve vs two 512-free scans. The act wait covers this
                # unit's stage AND the extract of unit u-2 (junk[h] WAR).
                nc.vector.wait_ge(pe_sem, 2 * u + 2)
                nc.vector.wait_ge(act_sem, n_act)
                nc.vector.tensor_tensor_scan(
                    out=junk[h][:, :],
                    data0=psum[:, base:base + 1024],
                    data1=scratch[h][:, :],
                    initial=FMAX,
                    op0=mybir.AluOpType.min, op1=mybir.AluOpType.min,
                ).then_inc(vec_sem, 1)

            for uu in (NUNITS3 - 2, NUNITS3 - 1):
                nc.scalar.wait_ge(vec_sem, uu + 1)
                emit_extract(uu)
                n_act += 1
            return n_act

        if loop_iters is None:
            total_act = emit_body()
            nc.sync.wait_ge(act_sem, total_act)
        else:
            with nc.Fori(0, loop_iters):
                emit_body()
                nc.all_engine_barrier()
                nc.vector.sem_clear(pe_sem)
                nc.vector.sem_clear(act_sem)
                nc.vector.sem_clear(vec_sem)
                nc.all_engine_barrier()

        nc.sync.dma_start(out=OUT[:], in_=mins[:]).then_inc(dma_sem, 16)
        nc.sync.wait_ge(dma_sem, 80)

    return nc


def _build_nc_v3(loop_iters: int | None = None):
    """v3 (fallback, measured 145.8us): PE streams one K=11 fp16 matmul per
    (tile, chunk) into a rotating half of PSUM (no strip rotation); ScalarE
    stages the odd bank-pair of each 4-bank unit to SBUF; VectorE runs one
    1024-free tensor_tensor_scan(min,min) over (2 PSUM banks, 2 staged
    banks) per unit; ScalarE extracts the scan tail.

    loop_iters: if set, wraps the compute body in an on-device Fori loop
    (with semaphore clears + engine barriers between iterations) for
    steady-state benchmarking. Results are identical."""
    import concourse.bass as bass
    from concourse import mybir
    from contextlib import ExitStack

    nc = bass.Bass("TRN2", target_bir_lowering=False, debug=False,
                   num_devices=NCORES)
    WR = nc.dram_tensor("WR", [KROWS, 2 * N], mybir.dt.float16,
                        kind="ExternalInput").ap()
    OUT = nc.dram_tensor("OUT", [128, NUNITS3], mybir.dt.float32,
                         kind="ExternalOutput").ap()

    ctx = ExitStack()
    with ctx:
        wr = ctx.enter_context(
            nc.sbuf_tensor("wr_sb", [KROWS, 2 * N], mybir.dt.float16)).ap()
        mins = ctx.enter_context(
            nc.sbuf_tensor("mins_sb", [128, NUNITS3], mybir.dt.float32)).ap()
        scratch = [
            ctx.enter_context(
                nc.sbuf_tensor(f"scr{i}", [128, 1024], mybir.dt.float32)).ap()
            for i in range(2)
        ]
        junk = [
            ctx.enter_context(
                nc.sbuf_tensor(f"junk{i}", [128, 1024], mybir.dt.float32)).ap()
            for i in range(2)
        ]
        psum = ctx.enter_context(
            nc.psum_tensor("psum", [128, N], mybir.dt.float32)).ap()
        dma_sem = ctx.enter_context(nc.semaphore("dma_sem"))
        pe_sem = ctx.enter_context(nc.semaphore("pe_sem"))
        act_sem = ctx.enter_context(nc.semaphore("act_sem"))
        vec_sem = ctx.enter_context(nc.semaphore("vec_sem"))

        def w_ap(t):
            return wr[0:11, t * 128:(t + 1) * 128]

        def r_ap(c):
            return wr[0:11, N + c * 512:N + (c + 1) * 512]

        nc.sync.dma_start(out=wr[:], in_=WR[:]).then_inc(dma_sem, 16)

        def emit_extract(uu):
            # ScalarE: scan uu's tail (last column of junk[uu%2]) -> mins.
            # Emitted right after the stage of unit uu+2, whose vec_sem wait
            # (>= uu+1) is exactly this op's dependency -- it never adds a
            # stall to the Act queue.
            nc.scalar.copy(mins[:, uu:uu + 1], junk[uu % 2][:, 1023:1024]
                           ).then_inc(act_sem, 1)

        def emit_body():
            nc.tensor.wait_ge(dma_sem, 16)
            n_act = 0
            for u in range(NUNITS3):
                t, h = u // 2, u % 2
                base = 2048 * h
                # Staged pair (banks 2,3) first: they only need the STAGE of
                # unit u-2 to have drained them (act), not its scan -- this
                # keeps the PE and ScalarE off each other's critical cycle.
                if u >= 2:
                    nc.tensor.wait_ge(act_sem,
                                      2 * u - 5 if u >= 4 else u - 1)
                for j in (2, 3):
                    mm = nc.tensor.matmul(
                        psum[:, base + 512 * j:base + 512 * (j + 1)],
                        w_ap(t), r_ap(4 * h + j), start=True, stop=True)
                mm.then_inc(pe_sem, 1)
                # PSUM pair (banks 0,1): consumed by scan u-2.
                if u >= 2:
                    nc.tensor.wait_ge(vec_sem, u - 1)
                for j in (0, 1):
                    mm = nc.tensor.matmul(
                        psum[:, base + 512 * j:base + 512 * (j + 1)],
                        w_ap(t), r_ap(4 * h + j), start=True, stop=True)
                mm.then_inc(pe_sem, 1)

                # ScalarE: banks {2,3} of the quad -> SBUF scratch
                nc.scalar.wait_ge(pe_sem, 2 * u + 1)
                if u >= 2:
                    nc.scalar.wait_ge(vec_sem, u - 1)  # scratch WAR vs scan
                nc.scalar.copy(scratch[h][:, :],
                               psum[:, base + 1024:base + 2048]
                               ).then_inc(act_sem, 1)
                n_act += 1
                if u >= 2:
                    emit_extract(u - 2)
                    n_act += 1

                # VectorE: running min over (psum-pair min staged-pair); the
                # unit's row-min lands in the last scan column of junk[h].
                # The act wait covers this unit's stage AND the extract of
                # unit u-2 (junk[h] WAR).
                nc.vector.wait_ge(pe_sem, 2 * u + 2)
                nc.vector.wait_ge(act_sem, n_act)
                nc.vector.tensor_tensor_scan(
                    out=junk[h][:, :],
                    data0=psum[:, base:base + 1024],
                    data1=scratch[h][:, :], initial=FMAX,
                    op0=mybir.AluOpType.min, op1=mybir.AluOpType.min,
                ).then_inc(vec_sem, 1)

            for uu in (NUNITS3 - 2, NUNITS3 - 1):
                nc.scalar.wait_ge(vec_sem, uu + 1)
                emit_extract(uu)
                n_act += 1
            return n_act

        if loop_iters is None:
            total_act = emit_body()
            nc.sync.wait_ge(act_sem, total_act)
        else:
            with nc.Fori(0, loop_iters):
                emit_body()
                nc.all_engine_barrier()
                nc.vector.sem_clear(pe_sem)
                nc.vector.sem_clear(act_sem)
                nc.vector.sem_clear(vec_sem)
                nc.all_engine_barrier()

        nc.sync.dma_start(out=OUT[:], in_=mins[:]).then_inc(dma_sem, 16)
        nc.sync.wait_ge(dma_sem, 32)

    return nc


def _pack_core_inputs_v3(P: np.ndarray, S: np.ndarray):
    """P: [N, 3] query points, S: [N, 3] database points.

    Returns WR fp16 [KROWS, 2N]: cols [:N] = stationary W, cols [N:] = R.
      W rows: [Ph.T(3), Ph.T(3), Pl.T(3), 1, 1, pad]
      R rows: [Uh.T(3), Ul.T(3), Uh.T(3), s2h, s2l, pad]  (U = -2 S)
    """
    f16 = np.float16
    P = P.astype(np.float32)
    S = S.astype(np.float32)

    Ph = P.astype(f16)
    Pl = (P - Ph.astype(np.float32)).astype(f16)
    U = -2.0 * S
    Uh = U.astype(f16)
    Ul = (U - Uh.astype(np.float32)).astype(f16)
    s2 = (S ** 2).sum(-1)
    s2h = s2.astype(f16)
    s2l = (s2 - s2h.astype(np.float32)).astype(f16)

    W = np.zeros((KROWS, N), f16)
    W[0:3] = Ph.T
    W[3:6] = Ph.T
    W[6:9] = Pl.T
    W[9] = 1.0
    W[10] = 1.0
    R = np.zeros((KROWS, N), f16)
    R[0:3] = Uh.T
    R[3:6] = Ul.T
    R[6:9] = Uh.T
    R[9] = s2h
    R[10] = s2l
    return np.ascontiguousarray(np.concatenate([W, R], axis=1))


def _unpack_mins(mins: np.ndarray) -> np.ndarray:
    """-> per-query min over db of (-2 q.s + ||s||^2), indexed by query n."""
    cols = 4 if VERSION == 5 else 2
    m = mins.reshape(128, NTILES, cols).min(axis=2)  # [p, t]
    return m.T.reshape(N)  # n = t*128 + p


def make_in_maps(set1: np.ndarray, set2: np.ndarray):
    """Per-core input maps + per-core query norms."""
    in_maps, qnorms = [], []
    for c in range(NCORES):
        b, ori = c // 2, c % 2
        P = set1[b] if ori == 0 else set2[b]
        S = set2[b] if ori == 0 else set1[b]
        WR = _pack_core_inputs_v3(P, S)
        in_maps.append({"WR": WR})
        qnorms.append((P.astype(np.float32) ** 2).sum(-1))
    return in_maps, qnorms


def _get_nc():
    global _nc_cache
    if _nc_cache is None:
        _nc_cache = {3: _build_nc_v3, 4: _build_nc_v4,
                     5: _build_nc_v5}[VERSION]()
    return _nc_cache


def kernel(set1: np.ndarray, set2: np.ndarray) -> np.ndarray:
    from concourse.bass_utils import run_bass_kernel_spmd

    set1 = np.asarray(set1, dtype=np.float32)
    set2 = np.asarray(set2, dtype=np.float32)

    nc = _get_nc()
    in_maps, qnorms = make_in_maps(set1, set2)
    res = run_bass_kernel_spmd(nc, in_maps, list(range(NCORES)))
    terms = []
    for c in range(NCORES):
        raw = _unpack_mins(np.asarray(res.results[c]["OUT"]))
        d2 = np.maximum(raw + qnorms[c], 0.0).astype(np.float32)
        terms.append(np.sqrt(d2).mean(dtype=np.float32))
    total = np.mean([terms[2 * b] + terms[2 * b + 1] for b in range(B)],
                    dtype=np.float32)
    return np.array(total, dtype=np.float32)
